# revision 53
# baseline (speedup 1.0000x reference)
"""Trainium2 Bass kernel for nn_Cross_attention_89730456748795.

Full-input contract: kernel(**inputs) takes the unsharded inputs of
reference.setup_inputs() and returns the full [4, 128, 64, 64] output.

Sharding: the model has 4 batches x 2 cross-attention branches = 8
independent attention instances; one per NeuronCore (pure data parallel,
no collectives). Each core:
  - runs both depthwise convs (3x3 reflect-pad-1 and 5x5 reflect-pad-2)
    as diagonal-weight PE matmuls over a reflect-padded image, with
    bias+LeakyReLU+per-channel sum fused into one Act op per tile and
    the sum-of-squares in a second Act op (accum_out),
  - folds both GroupNorms into the QKV projection weights (scale into the
    stationary operand, bias via a tiny N=1 matmul),
  - computes one full 4096x4096 attention in scores-transposed layout
    (softmax denominators come free from a ones-column appended to V^T);
    the exp over the 16.7M scores is split between the Act engine (true
    Exp) and the DVE (Schraudolph bit-trick exp: bf16 bits = int16(
    scores*23.083 + 16250.5), max weight err ~3%, cancels in softmax),
  - output-projects, divides by the softmax sums, adds bias + residual.
"""

import math

import numpy as np

B, C, H, W = 4, 64, 64, 64
HW = H * W  # 4096
PAD = 2
HP = H + 2 * PAD  # 68
N_CORES = 8
GROUPS = 16
EPS = 1e-5
NBLK = 8  # q blocks of 512
BLK = 512
NCHUNK = 32  # k chunks of 128
CHUNK = 128

# Schraudolph exp in fp8e4m3-bit space, with the 1/sqrt(C) score scale and
# a constant -2.0 shift (cancels in softmax) folded:
# e4m3_bits(exp(s/8 - 2)) ~= round(s * 0.125*log2(e)*8 + (7<<3) - 2*log2(e)*8 - 0.34)
# Output as uint8 so negative bit values saturate to 0 (= +0.0 weight).
SCH_A = 0.125 * 1.4426950408889634 * 8.0
SCH_B = 9.37
EXP_SHIFT = -4.0  # exp(s/8 - 4): keeps fp8 range safe (real s/8 hits 8.6);
# the constant shift cancels in the softmax ratio

# conv tap groups: (dy, dx0, paired). Pairs cover (dy,dx0),(dy,dx0+1).
TAP_GROUPS = [(dy, dx0, True) for dy in range(5) for dx0 in (0, 2)] + [
    (dy, 4, False) for dy in range(5)
]
NG = len(TAP_GROUPS)  # 15

_compiled = None
_last = None
TRACE = False  # dev-only: capture NTFF profile via run_bass_kernel_spmd


def _build(dbg=False):
    import concourse.tile as tile
    from concourse import bacc, mybir

    f32 = mybir.dt.float32
    f32r = mybir.dt.float32r
    bf16 = mybir.dt.bfloat16
    i16 = mybir.dt.int16
    u8 = mybir.dt.uint8
    f8 = mybir.dt.float8e4
    Alu = mybir.AluOpType
    Act = mybir.ActivationFunctionType
    DRow = mybir.MatmulPerfMode.DoubleRow

    nc = bacc.Bacc("TRN2", target_bir_lowering=False, debug=False,
                   num_devices=N_CORES)

    def din(name, shape, dt=f32):
        return nc.dram_tensor(name, shape, dt, kind="ExternalInput").ap()

    x_in = din("x_in", [128, HP * HP], bf16)  # top: padded, bottom: +1 shift
    conv_wt = din("conv_wt", [128, NG * 128], bf16)
    conv_b = din("conv_b", [128, 1])
    gn_w = din("gn_w", [128, 1])
    gn_b = din("gn_b", [128, 1])
    gmat = din("gmat", [128, 128])
    wkk = din("wkk", [128, 128])
    wqq = din("wqq", [128, 128])
    wvr = din("wvr", [128, 128])
    proj_b = din("proj_b", [C, 1])
    out_d = nc.dram_tensor("out", [C, HW], f32, kind="ExternalOutput").ap()
    dbg_d = {}
    if dbg:
        for nm, shp in [("d_xc", [128, HW]), ("d_kk", [128, HW // 2]),
                        ("d_qq", [128, HW // 2]), ("d_vres", [C, HW]),
                        ("d_vaug", [128, NCHUNK * 32]),
                        ("d_s1p", [128, NBLK // 2]),
                        ("d_s2p", [128, NBLK // 2]),
                        ("d_scale", [128, 1]), ("d_beff", [128, 1]),
                        ("d_et", [128, 2 * BLK])]:
            dbg_d[nm] = nc.dram_tensor(nm, shp, f32,
                                       kind="ExternalOutput").ap()

    with tile.TileContext(nc) as tc:
        # ---- persistent SBUF tensors ----
        persist = tc.alloc_tile_pool(name="persist", bufs=1)

        def T(shape, name, dt=f32):
            return persist.tile(shape, dt, tag=name, name=name)

        x2 = T([128, HP, HP], "x2", bf16)  # top: padded, bottom: +1 shift
        cw = T([128, NG * 128], "cw", bf16)
        xc = T([128, HW], "xc", bf16)  # conv+leaky out (br1|br2)
        kk = T([128, HW], "kk", bf16)  # kf duplicated on both halves
        qq = T([128, HW], "qq", bf16)  # qf duplicated on both halves
        vres = T([C, HW], "vres")  # residual rows only
        vaug = T([128, NCHUNK, 128], "vaug", f8)  # [.., t, 0:64]=V^T, 64=ones
        gmat_s = T([128, 128], "gmat_s")
        wkk_s = T([128, 128], "wkk_s")
        wqq_s = T([128, 128], "wqq_s")
        wvr_s = T([128, 128], "wvr_s")
        wkk_e = T([128, 128], "wkk_e", bf16)
        wqq_e = T([128, 128], "wqq_e", bf16)
        wvr_e = T([128, 128], "wvr_e", bf16)
        proj_b_s = T([C, 1], "proj_b_s")
        pb_eff = T([C, 1], "pb_eff")
        conv_b_s = T([128, 1], "conv_b_s")
        gn_w_s = T([128, 1], "gn_w_s")
        gn_b_s = T([128, 1], "gn_b_s")
        dump = T([128, 2 * BLK], "dump")
        s1p = T([128, NBLK // 2 + 1], "s1p")
        s2p = T([128, NBLK // 2 + 1], "s2p")
        stats = T([128, 2], "stats")
        mean_s = T([128, 1], "mean_s")
        var_s = T([128, 1], "var_s")
        vpe_s = T([128, 1], "vpe_s")
        half_s = T([128, 1], "half_s")
        y0_s = T([128, 1], "y0_s")
        ysq_s = T([128, 1], "ysq_s")
        pcor_s = T([128, 1], "pcor_s")
        c15_s = T([128, 1], "c15_s")
        rstd_s = T([128, 1], "rstd_s")
        scale_s = T([128, 1], "scale_s")
        negscale = T([128, 1], "negscale")
        beff = T([128, 1], "beff")
        bkk = T([128, 1], "bkk")
        bqq = T([128, 1], "bqq")
        bvr = T([128, 1], "bvr")
        eps_s = T([128, 1], "eps_s")
        shift_s = T([128, 1], "shift_s")

        sy = nc.sync

        # ---- input + constant DMAs. The host sends both the padded image
        # AND its +1-column-shifted copy as one bf16 [128, HP*HP] tensor,
        # so no on-chip shift pass is needed and all 128 partitions fill
        # in one descriptor per strip. Strips sized to conv-quad needs
        # (quad jq reads rows [16jq, 16jq+21)), spread across queues.
        x2f = x2[:].rearrange("p a b -> p (a b)")
        nc.scalar.dma_start(cw[:, 0:8 * 128], conv_wt[:, 0:8 * 128])
        STRIPS = [(0, 11), (11, 21), (21, 29), (29, 37),
                  (37, 45), (45, 53), (53, 61), (61, 68)]
        squeues = [sy, nc.gpsimd]
        for i, (r0, r1) in enumerate(STRIPS):
            a, b = r0 * HP, r1 * HP
            squeues[i % 2].dma_start(x2f[:, a:b], x_in[:, a:b])
        nc.scalar.dma_start(cw[:, 8 * 128:], conv_wt[:, 8 * 128:])
        nc.scalar.dma_start(conv_b_s[:], conv_b[:])
        nc.scalar.dma_start(gmat_s[:], gmat[:])
        nc.scalar.dma_start(wkk_s[:], wkk[:])
        nc.scalar.dma_start(wqq_s[:], wqq[:])
        nc.scalar.dma_start(wvr_s[:], wvr[:])
        sy.dma_start(proj_b_s[:], proj_b[:])
        sy.dma_start(gn_w_s[:], gn_w[:])
        sy.dma_start(gn_b_s[:], gn_b[:])
        nc.vector.memset(vaug[:], 1.0)
        nc.vector.memset(eps_s[:], EPS)
        nc.vector.memset(c15_s[:], 1.5)
        nc.vector.memset(shift_s[:], EXP_SHIFT)

        # ---- depthwise convs as diagonal matmuls; bias+leaky+sum fused in
        # one Act op per 1024-wide pair, sum-of-squares in a second ----
        with tc.tile_pool(name="cvp", bufs=2, space="PSUM") as pps:
            for jq in range(NBLK // 2):
                cps = pps.tile([128, 2, BLK], f32, tag="conv",
                               name=f"cps{jq}")
                for g, (dy, dx0, paired) in enumerate(TAP_GROUPS):
                    for i in range(2):
                        j = jq * 2 + i
                        lhs = cw[:, g * 128:(g + 1) * 128]
                        rows = slice(dy + 8 * j, dy + 8 * j + 8)
                        rhs = x2[:, rows, dx0:dx0 + W]
                        if not paired:
                            lhs = cw[0:C, g * 128:(g + 1) * 128]
                            rhs = x2[0:C, rows, dx0:dx0 + W]
                        nc.tensor.matmul(cps[:, i, :], lhs, rhs,
                                         start=(g == 0), stop=(g == NG - 1))
                # last quad's evacuation split in halves so the final
                # stats column lands ~1.1us earlier (shorter GN stall)
                if jq < NBLK // 2 - 1:
                    parts = [(slice(0, 2), slice(jq * 2 * BLK,
                                                 (jq + 1) * 2 * BLK), jq)]
                else:
                    parts = [
                        (slice(0, 1), slice(jq * 2 * BLK,
                                            jq * 2 * BLK + BLK), jq),
                        (slice(1, 2), slice(jq * 2 * BLK + BLK,
                                            (jq + 1) * 2 * BLK), jq + 1),
                    ]
                for hsl, qslc, scol in parts:
                    cflat = cps[:, hsl, :].rearrange("p a b -> p (a b)")
                    nc.scalar.activation(
                        xc[:, qslc], cflat, Act.Lrelu,
                        bias=conv_b_s[:, 0:1], alpha=0.01,
                        accum_out=s1p[:, scol:scol + 1])
                    # sum-of-squares on DVE so the last quad's stats tail
                    # overlaps the Act LeakyReLU instead of queueing
                    xb = xc[:, qslc]
                    nc.vector.scalar_tensor_tensor(
                        out=dump[:, 0:(qslc.stop - qslc.start)], in0=xb,
                        scalar=1.0, in1=xb,
                        op0=Alu.mult, op1=Alu.mult,
                        accum_out=s2p[:, scol:scol + 1])

        # ---- group-norm statistics (fold into projection weights) ----
        with tc.tile_pool(name="stp", bufs=2, space="PSUM") as pps:
            # keep the PE streaming through the ~6.5us stats/fold stall:
            # an idle PE drops to the 1.2GHz mid p-state and the whole
            # projection phase then crawls until 3us of continuous work
            warm = pps.tile([128, BLK], f32, tag="warm", bufs=1)
            for wi in range(20):
                nc.tensor.matmul(warm[:], cw[:, 0:128],
                                 x2[:, (wi % 16):(wi % 16) + 8, 0:W])
            nc.vector.tensor_copy(dump[:, 0:BLK], warm[:])
            nc.vector.tensor_reduce(stats[:, 0:1], s1p[:, 0:5],
                                    axis=mybir.AxisListType.X, op=Alu.add)
            nc.vector.tensor_reduce(stats[:, 1:2], s2p[:, 0:5],
                                    axis=mybir.AxisListType.X, op=Alu.add)
            gps = pps.tile([128, 2], f32, tag="gstat", bufs=1)
            nc.tensor.matmul(gps[:], gmat_s[:], stats[:, 0:2])
            # negvar = mean^2 - m2; rstd = rsqrt(var+eps) via the bit-trick
            # initial guess + 2 Newton steps, all on DVE: avoids Ln/Exp here
            # so every Act function in the kernel (Lrelu/Identity/Exp) lives
            # in one act table -> no mid-kernel ACT_TABLE_LOADs.
            i32 = mybir.dt.int32
            nc.vector.tensor_copy(mean_s[:], gps[:, 0:1])
            nc.vector.scalar_tensor_tensor(
                out=var_s[:], in0=mean_s[:], scalar=mean_s[:, 0:1],
                in1=gps[:, 1:2], op0=Alu.mult, op1=Alu.subtract)
            # vpe = var + eps = -var_s + eps ; half = -0.5 * vpe
            nc.vector.tensor_scalar(
                out=vpe_s[:], in0=var_s[:], scalar1=-1.0,
                scalar2=eps_s[:, 0:1], op0=Alu.mult, op1=Alu.add)
            nc.vector.tensor_scalar_mul(half_s[:], vpe_s[:], -0.5)
            # y0 = bitcast(0x5f3759df - (bits(vpe) >> 1))
            nc.vector.tensor_scalar(
                out=y0_s[:].bitcast(i32), in0=vpe_s[:].bitcast(i32),
                scalar1=1, scalar2=None, op0=Alu.arith_shift_right)
            nc.vector.tensor_scalar(
                out=y0_s[:].bitcast(i32), in0=y0_s[:].bitcast(i32),
                scalar1=-1, scalar2=0x5F3759DF, op0=Alu.mult, op1=Alu.add)
            for _ in range(1):  # Newton: y = y * (1.5 + (-0.5*vpe) * y^2)
                nc.vector.tensor_mul(ysq_s[:], y0_s[:], y0_s[:])
                nc.vector.scalar_tensor_tensor(
                    out=pcor_s[:], in0=ysq_s[:], scalar=half_s[:, 0:1],
                    in1=c15_s[:], op0=Alu.mult, op1=Alu.add)
                nc.vector.tensor_mul(y0_s[:], y0_s[:], pcor_s[:])
            nc.vector.tensor_copy(rstd_s[:], y0_s[:])
            nc.vector.tensor_mul(scale_s[:], rstd_s[:], gn_w_s[:])
            nc.vector.tensor_scalar_mul(negscale[:], scale_s[:], -1.0)

            # fold GN scale into k/q weights FIRST — the projection matmuls
            # wait only on these; the bias matmuls (Act-side operands) and
            # the v fold can trail
            nc.vector.tensor_scalar_mul(wkk_e[:], wkk_s[:], scale_s[:, 0:1])
            nc.vector.tensor_scalar_mul(wqq_e[:], wqq_s[:], scale_s[:, 0:1])
            nc.vector.scalar_tensor_tensor(
                out=beff[:], in0=mean_s[:], scalar=negscale[:, 0:1],
                in1=gn_b_s[:], op0=Alu.mult, op1=Alu.add)
            nc.vector.tensor_scalar_mul(wvr_e[:], wvr_s[:], scale_s[:, 0:1])
            for wsb, bsb in ((wkk_s, bkk), (wqq_s, bqq), (wvr_s, bvr)):
                bps = pps.tile([128, 1], f32, tag="bias")
                nc.tensor.matmul(bps[:], wsb[:], beff[:])
                nc.vector.tensor_copy(bsb[:], bps[:])
            # v-channel GN bias + projection bias ride through the softmax;
            # fold both + the residual GN bias into vres's per-channel bias
            # so the attention tail is a plain tensor_add (Pool-capable)
            nc.vector.tensor_copy(pb_eff[:], bvr[C:128, 0:1])
            nc.vector.tensor_add(pb_eff[:], pb_eff[:], proj_b_s[:])
            nc.vector.tensor_add(pb_eff[:], pb_eff[:], bvr[0:C, 0:1])

        # ---- k/q/v projections + V^T transposes. k/q evacuation on Act,
        # v + transposes on DVE, so no engine serializes the prefix ----
        with tc.tile_pool(name="vrp", bufs=2, space="PSUM") as pps, \
             tc.tile_pool(name="trp", bufs=2, space="PSUM") as tpp:
            for jq in range(NBLK // 2):
                blk2 = slice(jq * 2 * BLK, (jq + 1) * 2 * BLK)
                # V^T tiles first: their small DVE casts must precede the
                # big q-evacuation in the DVE queue, else the 2-deep tps
                # PSUM pool stalls the PE between transposes
                for t in range(8 * jq, 8 * jq + 8):
                    tps = tpp.tile([128, C], f32, tag="tr")
                    nc.tensor.matmul(tps[:], xc[:, t * 128:(t + 1) * 128],
                                     wvr_e[:, C:128])
                    nc.vector.tensor_copy(vaug[:, t, 0:C], tps[:])
                kps = pps.tile([128, 2 * BLK], f32, tag="kq")
                nc.tensor.matmul(kps[:, 0:BLK], wkk_e[:],
                                 xc[:, jq * 2 * BLK:jq * 2 * BLK + BLK])
                nc.tensor.matmul(kps[:, BLK:], wkk_e[:],
                                 xc[:, jq * 2 * BLK + BLK:(jq + 1) * 2 * BLK])
                nc.scalar.activation(kk[:, blk2], kps[:], Act.Identity,
                                     bias=bkk[:, 0:1])
                qps = pps.tile([128, 2 * BLK], f32, tag="kq")
                nc.tensor.matmul(qps[:, 0:BLK], wqq_e[:],
                                 xc[:, jq * 2 * BLK:jq * 2 * BLK + BLK])
                nc.tensor.matmul(qps[:, BLK:], wqq_e[:],
                                 xc[:, jq * 2 * BLK + BLK:(jq + 1) * 2 * BLK])
                # q evacuation on DVE (after this quad's casts in the DVE
                # queue); vres evacuation on Act — balances Act ~2.2us,
                # DVE ~2.6us, PE ~3us per quad
                nc.vector.tensor_scalar(
                    out=qq[:, blk2], in0=qps[:], scalar1=bqq[:, 0:1],
                    scalar2=None, op0=Alu.add)
                for i in range(2):
                    bsl = slice(jq * 2 * BLK + i * BLK,
                                jq * 2 * BLK + (i + 1) * BLK)
                    vps = pps.tile([C, BLK], f32, tag="vv")
                    nc.tensor.matmul(vps[:], wvr_e[:, 0:C], xc[:, bsl])
                    nc.scalar.activation(vres[:, bsl], vps[:], Act.Identity,
                                         bias=pb_eff[0:C, 0:1])

        if dbg:
            sy.dma_start(dbg_d["d_xc"][:], xc[:].bitcast(f32))
            sy.dma_start(dbg_d["d_kk"][:], kk[:].bitcast(f32))
            sy.dma_start(dbg_d["d_qq"][:], qq[:].bitcast(f32))
            sy.dma_start(dbg_d["d_vres"][:], vres[:])
            sy.dma_start(dbg_d["d_vaug"][:], vaug[:].bitcast(f32))
            sy.dma_start(dbg_d["d_s1p"][:], s1p[:])
            sy.dma_start(dbg_d["d_s2p"][:], s2p[:])
            sy.dma_start(dbg_d["d_scale"][:], scale_s[:])
            sy.dma_start(dbg_d["d_beff"][:], beff[:])

        # ---- attention: per q-block, 16 chunk-pairs; the scores PSUM
        # rotates 3-deep and stq emission runs two pairs ahead of AV so
        # the PE never queues behind an exp wait. exp alternates between
        # Act (true Exp) and DVE (Schraudolph bit-trick) ----
        with tc.tile_pool(name="aps", bufs=3, space="PSUM") as aps, \
             tc.tile_pool(name="ops", bufs=2, space="PSUM") as ops, \
             tc.tile_pool(name="asb", bufs=2) as asb:
            NP = NCHUNK // 2  # 16 chunk-pairs per q-block
            NGP = NBLK * NP  # 128 global pairs — one flat stream, so the
            # next block's scores interleave with this block's last AVs
            # and the PE never sees a block boundary
            ets = {}
            obs = {}

            def emit_stq(gp):
                j, cp = divmod(gp, NP)
                qs = slice(j * BLK, (j + 1) * BLK)
                c0, c1 = 2 * cp, 2 * cp + 1
                k0 = slice(c0 * CHUNK, (c0 + 1) * CHUNK)
                k1 = slice(c1 * CHUNK, (c1 + 1) * CHUNK)
                stq = aps.tile([128, 2 * BLK], f32, tag="stq")
                # even chunk on PE rows 0-63, odd on rows 64-127
                nc.tensor.matmul(stq[:, 0:BLK],
                                 kk[0:C, k0], qq[0:C, qs])
                nc.tensor.matmul(stq[:, BLK:2 * BLK],
                                 kk[C:128, k1], qq[C:128, qs])
                et = asb.tile([128, 2 * BLK], f8, tag="expst", bufs=8)
                if cp % 2 == 0 or cp == NP - 1:
                    nc.scalar.activation(et[:], stq[:], Act.Exp,
                                         scale=0.125,
                                         bias=shift_s[:, 0:1])
                else:
                    nc.vector.tensor_scalar(
                        out=et[:].bitcast(u8), in0=stq[:],
                        scalar1=SCH_A, scalar2=SCH_B,
                        op0=Alu.mult, op1=Alu.add)
                ets[gp] = et

            def emit_av(gp):
                j, cp = divmod(gp, NP)
                if cp == 0:
                    obs[j] = ops.tile([65, BLK], f32, tag="oacc",
                                      name=f"o{j}")
                et = ets.pop(gp)
                # fp8 DoubleRow: both 128-key chunks contract in one
                # matmul at 2 rows/cycle (0.5 cyc per output column)
                et3 = et[:].rearrange("p (t n) -> p t n", t=2)
                va3 = vaug[:, 2 * cp:2 * cp + 2, 0:65]
                nc.tensor.matmul(obs[j][:], va3, et3,
                                 start=(cp == 0), stop=(cp == NP - 1),
                                 perf_mode=DRow)

            def make_tail(j, nh):
                # tail: normalize via reciprocal of the ones-column sums,
                # + residual (biases folded into vres), ship out. Returned
                # as single-op closures the main loop paces one per pair,
                # so the tail never bursts onto an engine the exp
                # pipeline needs.
                ob = obs.pop(j)
                hw_ = BLK // nh
                steps = []
                state = {}
                for h in range(nh):
                    cs = slice(h * hw_, (h + 1) * hw_)
                    gs = slice(j * BLK + h * hw_, j * BLK + (h + 1) * hw_)

                    def s1(h=h, cs=cs):
                        # evacuate PSUM via cheap copies first — DVE math
                        # on PSUM operands runs ~4x slower on HW
                        osb = asb.tile([C, hw_], f32, tag=f"osb{h % 2}")
                        nc.scalar.copy(osb[:], ob[0:C, cs])
                        ss = asb.tile([1, hw_], f32, tag=f"ss{h % 2}")
                        nc.vector.tensor_copy(ss[:], ob[C:C + 1, cs])
                        state[("o", h)] = osb
                        state[("s", h)] = ss

                    def s2(h=h):
                        rs = asb.tile([1, hw_], f32, tag=f"rs{h % 2}")
                        nc.vector.reciprocal_approx_fast(
                            rs[:], state.pop(("s", h))[:])
                        rbs = asb.tile([C, hw_], f32, tag=f"rbs{h % 2}")
                        nc.gpsimd.partition_broadcast(rbs[:], rs[:])
                        state[("r", h)] = rbs

                    def s3(h=h):
                        tmp = asb.tile([C, hw_], f32, tag=f"tmp{h % 2}")
                        nc.vector.tensor_mul(tmp[:], state.pop(("o", h))[:],
                                             state.pop(("r", h))[:])
                        state[("t", h)] = tmp

                    def s4(h=h, gs=gs):
                        ot = asb.tile([C, hw_], f32, tag=f"ot{h % 2}")
                        nc.vector.tensor_add(ot[:], state.pop(("t", h))[:],
                                             vres[0:C, gs])
                        sy.dma_start(out_d[:, gs], ot[:])

                    steps += [s1, s2, s3, s4]
                return steps

            for w in range(4):
                emit_stq(w)
            for gp in range(NGP):
                if gp + 4 < NGP:
                    emit_stq(gp + 4)
                emit_av(gp)
                j, cp = divmod(gp, NP)
                # previous block's tail goes in after this block's exp
                # pipeline is warmed, so it can't stall the first AVs
                if cp == 10 and j > 0:
                    for st in make_tail(j - 1, 1):
                        st()
            for st in make_tail(NBLK - 1, 4):
                st()

        persist.release()

    nc.compile()
    return nc


def _host_prep(inputs):
    """Build per-core input maps from the full inputs."""
    import ml_dtypes
    bf16 = ml_dtypes.bfloat16
    x = np.ascontiguousarray(inputs["inputs"], np.float32)  # [B, C, H, W]
    dw1 = np.asarray(inputs["dw1_w"], np.float32).reshape(C, 3, 3)
    dw2 = np.asarray(inputs["dw2_w"], np.float32).reshape(C, 5, 5)
    w3e = np.zeros((C, 5, 5), np.float32)
    w3e[:, 1:4, 1:4] = dw1
    conv_wt = np.zeros((128, NG, 128), np.float32)
    cidx = np.arange(C)
    for g, (dy, dx0, paired) in enumerate(TAP_GROUPS):
        conv_wt[cidx, g, cidx] = w3e[:, dy, dx0]
        conv_wt[cidx, g, C + cidx] = dw2[:, dy, dx0]
        if paired:
            conv_wt[C + cidx, g, cidx] = w3e[:, dy, dx0 + 1]
            conv_wt[C + cidx, g, C + cidx] = dw2[:, dy, dx0 + 1]
    conv_wt = conv_wt.reshape(128, NG * 128).astype(bf16)

    conv_b = np.concatenate([inputs["dw1_b"], inputs["dw2_b"]]).astype(
        np.float32).reshape(128, 1)
    gn_w = np.concatenate([inputs["gnA_w"], inputs["gnB_w"]]).astype(
        np.float32).reshape(128, 1)
    gn_b = np.concatenate([inputs["gnA_b"], inputs["gnB_b"]]).astype(
        np.float32).reshape(128, 1)

    gmat = np.zeros((128, 128), np.float32)
    cpg = C // GROUPS  # 4
    npix = cpg * HW
    for k in range(128):
        g0 = k // cpg
        gmat[k, g0 * cpg:(g0 + 1) * cpg] = 1.0 / npix
    gmat = gmat.T.copy()  # lhsT[k, m]: symmetric anyway, but be explicit

    qkvA = np.asarray(inputs["qkvA_w"], np.float32)
    qkvB = np.asarray(inputs["qkvB_w"], np.float32)

    in_maps = []
    xpads = {}
    for b in range(B):
        xp = np.pad(x[b], ((0, 0), (PAD, PAD), (PAD, PAD)), mode="reflect")
        x2 = np.zeros((128, HP, HP), np.float32)
        x2[0:C] = xp
        x2[C:128, :, 0:HP - 1] = xp[:, :, 1:HP]  # +1 column shift
        xpads[b] = np.ascontiguousarray(x2.reshape(128, HP * HP)).astype(bf16)
    for core in range(N_CORES):
        b, br = core // 2, core % 2
        wkk = np.zeros((128, 128), np.float32)
        wqq = np.zeros((128, 128), np.float32)
        wvr = np.zeros((128, 128), np.float32)
        if br == 0:  # out_A: k,v from branch1 (x_A), q from branch2 (x_B)
            wkk[0:C, 0:C] = qkvA[C:2 * C, :].T
            wkk[0:C, C:128] = qkvA[C:2 * C, :].T
            wqq[C:128, 0:C] = qkvB[0:C, :].T
            wqq[C:128, C:128] = qkvB[0:C, :].T
            wvr[0:C, 0:C] = np.eye(C)  # residual = x_A
            # fold the output projection into V: AV matmul then yields
            # the already-projected attention output
            pw, pb = inputs["outA_w"], inputs["outA_b"]
            wvr[0:C, C:128] = (np.asarray(pw, np.float64) @
                               np.asarray(qkvA[2 * C:3 * C, :], np.float64)).T
        else:  # out_B: k,v from branch2 (x_B), q from branch1 (x_A)
            wkk[C:128, 0:C] = qkvB[C:2 * C, :].T
            wkk[C:128, C:128] = qkvB[C:2 * C, :].T
            wqq[0:C, 0:C] = qkvA[0:C, :].T
            wqq[0:C, C:128] = qkvA[0:C, :].T
            wvr[C:128, 0:C] = np.eye(C)  # residual = x_B
            pw, pb = inputs["outB_w"], inputs["outB_b"]
            wvr[C:128, C:128] = (np.asarray(pw, np.float64) @
                                 np.asarray(qkvB[2 * C:3 * C, :], np.float64)).T
        in_maps.append({
            "x_in": xpads[b],
            "conv_wt": conv_wt,
            "conv_b": conv_b,
            "gn_w": gn_w,
            "gn_b": gn_b,
            "gmat": gmat,
            "wkk": wkk,
            "wqq": wqq,
            "wvr": wvr,
            "proj_b": np.asarray(pb, np.float32).reshape(C, 1),
        })
    return in_maps


def kernel(**inputs) -> np.ndarray:
    global _compiled, _last
    from concourse.bass_utils import run_bass_kernel_spmd

    if _compiled is None:
        _compiled = _build()
    in_maps = _host_prep(inputs)
    res = run_bass_kernel_spmd(_compiled, in_maps, core_ids=list(range(N_CORES)),
                               trace=TRACE)
    _last = res
    out = np.empty((B, 2 * C, H, W), np.float32)
    for core in range(N_CORES):
        b, br = core // 2, core % 2
        out[b, br * C:(br + 1) * C] = res.results[core]["out"].reshape(C, H, W)
    return out

